# revision 27
# baseline (speedup 1.0000x reference)
"""Trainium2 Bass kernel for nn_AdaptiveEpisodicMemory (scatter_memory).

Computes, for B=4096 queries over an M=65536-slot memory bank:

    scores = q @ K^T + 0.5 * c @ CTX^T + 0.3*exp(-0.1*(1-t))  (masked by used_slots)
    out    = softmax(scores) @ V

Strategy (8 NeuronCores):
  * Unused slots receive large-negative scores; their softmax weight is
    negligible, so the host drops them up-front (exact transformation) and
    pads the survivors to a multiple of 8*128. Shapes are chosen per-input
    at build time, so the kernel is correct for any input.
  * The memory bank (keys/contexts/values) is sharded across the 8 cores;
    query/context are replicated. The per-slot time-decay bias (and the -30
    pad mask) is folded into the score matmul as contraction row 96
    (qc row 96 = 1.0), so scores leave the PE fully biased.
  * Per core, sweeping 1024-query passes over m-tiles of 128 slots:
        S^T[m, b] = KC_shard^T.T @ QC^T     (two 512-col bf16 matmuls)
        P^T[m, b] = exp(S^T)                (exact exp on ScalarE for batch
                                             cols [0:512), bf16 bit-trick on
                                             VectorE for [512:1024) - both
                                             engines run every tile, halving
                                             per-tile exp latency)
        O[b, 65] += P_chunk.T @ Vaug        (FLIPPED: each 128-query P chunk
                                             is the stationary operand and
                                             the 65-wide Vaug = [V | 1]
                                             streams; measured ~29ns per
                                             chunk-matmul with the 128-col
                                             LDWEIGHTS fully hidden by the
                                             PE's per-subarray load/compute
                                             concurrency. Col 64 accumulates
                                             the softmax denominator.)
    The 8 chunk accumulators are [128, 65] psum regions packed 4 to a bank;
    a per-pass dummy matmul claims each bank (start=True clears has_written
    bank-wide, so the real chunk matmuls must all use start=False). O-matmul
    emission runs 2 tiles behind the score matmuls so their exp semaphores
    are satisfied at dispatch (the PE's unsatisfied-dep wait queue is only 4
    deep; a blocked O-matmul would head-of-line stall ready score matmuls).
  * Per pass the [batch, 65] partials DMA straight out; the host sums the
    8 cores' partials and divides by the denominator column. No device
    collectives, no device finale.
"""
import sys

sys.path.insert(0, "/opt/trn_rl_repo")
import math

import ml_dtypes
import numpy as np

from concourse import bass, bass_utils, mybir, tile

B, M, D, CD = 4096, 65536, 64, 32
KDIM = D + CD  # 96: contraction rows of the fused score matmul (bias row 96)
KPAD = 128  # padded to 128 so weight loads take the fast path
VAW = 128  # Vaug padded from 65 to 128 columns in storage (DMA alignment)
OW = D + 1  # 65: value columns + denominator column
NCORES = 8
BCHUNK = 512
PASSW = 1024  # batch width per pass
CPP = PASSW // BCHUNK
SCOL = 528  # exp split: ScalarE exact on [0:SCOL), VectorE trick on the rest
F32 = mybir.dt.float32
BF16 = mybir.dt.bfloat16
I16 = mybir.dt.int16
TIME_WEIGHT = 0.1
CURRENT_TIME = 1.0
DECAY_COEF = 0.3
NEG_PAD = -30.0  # pad-slot bias: e^-30 ~ 1e-13, vanishes vs real weights,
#                  and (unlike -1e9) stays in-range for the bit-trick exp
N_WARMUP_MM = 7
# bf16 bit-trick exp: bf16bits(e^x) ~ round(x * 128/ln2 + (127*128 - 5.5))
A_TRICK = 128.0 / math.log(2.0)
B_TRICK = 127.0 * 128.0 - 5.5


def _split_multi_waits(nc) -> int:
    """This walrus build accepts at most one fused sync-wait per instruction;
    hoist extras into standalone InstEventSemaphore instructions."""
    n_split = 0
    for fn in nc.m.functions:
        for bb in fn.blocks:
            insts = list(bb.instructions)
            out = []
            changed = False
            for inst in insts:
                si = inst.sync_info
                if si is not None and si.on_wait is not None and len(si.on_wait) > 1:
                    waits = list(si.on_wait)
                    for w in waits[:-1]:
                        ev = mybir.InstEventSemaphore(
                            name=f"{inst.name}-wsplit{n_split}",
                            engine=inst.engine,
                            ins=[],
                            outs=[],
                            sync_info=mybir.SyncInfo(on_wait=[w], on_update=[]),
                            bass_nofuse=True,
                        )
                        out.append(ev)
                        n_split += 1
                    inst.sync_info = mybir.SyncInfo(
                        on_wait=[waits[-1]], on_update=list(si.on_update or [])
                    )
                    changed = True
                out.append(inst)
            if changed:
                bb.instructions[:] = out
    return n_split


def _strip_flip_updates(nc) -> int:
    """Each PE instruction's completion bumps the PE semaphore (+1), and the
    ~14ns sem-send is paid on the engine - 1024 flip O-matmuls make that
    ~14us. Runs of consecutive 65-col flip matmuls never have mid-run
    consumers, so drop the increment from all but the last matmul of each
    run and renumber every wait threshold on that semaphore (thresholds
    that would land mid-run round UP to the run's final matmul - strictly
    conservative, so no ordering is ever weakened)."""

    def is_flip(inst):
        if not isinstance(inst, mybir.InstMatmult) or inst.is_transpose:
            return False
        si = inst.sync_info
        if si is None or len(si.on_update or []) != 1:
            return False
        u = si.on_update[0]
        if u.update_mode != "sem-inc" or u.update_value != 1:
            return False
        try:
            ap = inst.ins[0].ap
        except Exception:
            return False
        return any(pair[1] == 65 for pair in ap)

    total = 0
    for fn in nc.m.functions:
        all_insts = [i for bb in fn.blocks for i in bb.instructions]
        pe_insts = [
            i for i in all_insts
            if getattr(i, "engine", None) == mybir.EngineType.PE
            and not isinstance(i, mybir.InstLdweights)
        ]
        # the PE completion counter: the sem id every matmul bumps
        sem_pe = None
        for i in pe_insts:
            if isinstance(i, mybir.InstMatmult) and i.sync_info:
                for u in i.sync_info.on_update or []:
                    sem_pe = u.id
                    break
            if sem_pe is not None:
                break
        if sem_pe is None:
            continue
        # collect, in PE program order across blocks, every instruction
        # that updates sem_pe, marking flip-run members (all but the run's
        # last) as stripped
        upd_seq = []  # (inst, stripped)
        run_idx = []  # indices into upd_seq of the current flip run
        for inst in pe_insts + [None]:
            flip = inst is not None and is_flip(inst)
            if not flip and run_idx:
                for j in run_idx[:-1]:
                    upd_seq[j] = (upd_seq[j][0], True)
                run_idx = []
            if inst is None:
                break
            si = inst.sync_info
            has_upd = si is not None and any(
                u.id == sem_pe for u in (si.on_update or [])
            )
            if has_upd:
                upd_seq.append((inst, False))
                if flip:
                    run_idx.append(len(upd_seq) - 1)
        # old->new threshold map (1-based counts); a threshold landing on a
        # stripped update rounds UP to the run's surviving last update
        nupd = len(upd_seq)
        new_at = [0] * (nupd + 1)
        c_new = 0
        for k in range(1, nupd + 1):
            if not upd_seq[k - 1][1]:
                c_new += 1
            new_at[k] = c_new
        nxt = c_new
        for k in range(nupd, 0, -1):
            if upd_seq[k - 1][1]:
                new_at[k] = nxt
            else:
                nxt = new_at[k]
        # rewrite every wait on sem_pe, on every engine, in every block
        for inst in all_insts:
            si = inst.sync_info
            if si is None or not si.on_wait:
                continue
            changed = False
            ws = []
            for w in si.on_wait:
                if (
                    w.sync_type == "semaphore"
                    and w.id == sem_pe
                    and w.wait_mode == "sem-ge-imm"
                    and w.wait_value is not None
                    and 1 <= w.wait_value <= nupd
                ):
                    nv = new_at[w.wait_value]
                    if nv != w.wait_value:
                        w = mybir.SyncWait(
                            sync_type=w.sync_type,
                            id=w.id,
                            ant_name=w.ant_name,
                            wait_mode=w.wait_mode,
                            wait_value=nv,
                            wait_reg=w.wait_reg,
                        )
                        changed = True
                ws.append(w)
            if changed:
                inst.sync_info = mybir.SyncInfo(
                    on_wait=ws, on_update=list(si.on_update or [])
                )
        # finally strip the increments
        for inst, stripped in upd_seq:
            if stripped:
                inst.sync_info = mybir.SyncInfo(
                    on_wait=list(inst.sync_info.on_wait or []), on_update=[]
                )
                total += 1
    return total


def _build(m_loc: int):
    """Build the per-core Bass program for a shard of m_loc memory slots."""
    ntiles = m_loc // 128
    npass = B // PASSW
    nblk = PASSW // 128  # 128-query output chunks per pass
    nc = bass.Bass(trn_type="TRN2", debug=False, num_devices=NCORES)

    qc_ext = nc.dram_tensor("qc_t", [KPAD, B], BF16, kind="ExternalInput")
    kc_ext = nc.dram_tensor("kc_t", [KPAD, m_loc], BF16, kind="ExternalInput")
    # vaug arrives pre-arranged tile-major: [128, ntiles*VAW]
    va_ext = nc.dram_tensor("vaug", [128, ntiles * VAW], BF16, kind="ExternalInput")
    out_ext = nc.dram_tensor("out", [B // 128, 128, OW], F32, kind="ExternalOutput")

    with tile.TileContext(nc) as tc:
        with (
            tc.tile_pool(name="big", bufs=1) as big,
            tc.tile_pool(name="small", bufs=1) as small,
            tc.tile_pool(name="pT", bufs=6) as pTp,
            tc.tile_pool(name="psS", bufs=3, space="PSUM") as psS,
            tc.tile_pool(name="psF", bufs=1, space="PSUM") as psF,
            tc.tile_pool(name="fin", bufs=2) as fin,
        ):
            # PE warmup: keep TensorE busy from t=0 so HAM reaches 2.4 GHz
            # before the real matmuls start (inputs are still DMAing in).
            # GpSimd wakes earliest, so it seeds the warmup operand and the
            # dummy-exp input.
            wsrc = small.tile([128, 512], BF16)
            nc.gpsimd.memset(wsrc[:], 1.0)
            zt = small.tile([128, 128], BF16)
            nc.gpsimd.memset(zt[:], 0.0)
            dume_in = small.tile([128, 1], F32)
            nc.gpsimd.memset(dume_in[:], 0.0)
            wps = psS.tile(
                [128, 512], F32, name="wps", tag="sps", padded_shape=[128, PASSW]
            )
            for _ in range(N_WARMUP_MM):
                nc.tensor.matmul(
                    wps[:], lhsT=wsrc[:, 0:128], rhs=wsrc[:], start=True, stop=True
                )
            # big inputs, chunked and interleaved in rough consumption order:
            # the loop sweeps kc/va tiles k=0..ntiles-1 within pass 0 (which
            # reads qc columns [0, PASSW)) first. All triggers on the Sync
            # queue: the ~0.65us per-trigger issue cost paces the transfers
            # so the first (critical) pieces get near-exclusive bandwidth.
            qc_s = big.tile([KPAD, B], BF16)
            kc_s = big.tile([KPAD, m_loc], BF16)
            va_s = big.tile([128, ntiles * VAW], BF16)

            def _chunks(total, first):
                """[0:first], then ~512-col pieces covering the rest."""
                cuts = [0, min(first, total)]
                while cuts[-1] < total:
                    cuts.append(min(cuts[-1] + 512, total))
                return list(zip(cuts, cuts[1:]))

            qcp = _chunks(B, 512)
            kcp = _chunks(m_loc, 128)
            vap = _chunks(ntiles * VAW, 128)
            pieces = [(qc_s, qc_ext, *qcp[0]), (kc_s, kc_ext, *kcp[0]),
                      (qc_s, qc_ext, *qcp[1]), (va_s, va_ext, *vap[0]),
                      (kc_s, kc_ext, *kcp[1]), (va_s, va_ext, *vap[1])]
            order = []  # interleave kc/va (pass-0 critical) ahead of late qc
            ki, vi, qi = 2, 2, 2
            while ki < len(kcp) or vi < len(vap) or qi < len(qcp):
                if ki < len(kcp):
                    order.append((kc_s, kc_ext, *kcp[ki])); ki += 1
                if vi < len(vap):
                    order.append((va_s, va_ext, *vap[vi])); vi += 1
                if qi < len(qcp):
                    order.append((qc_s, qc_ext, *qcp[qi])); qi += 1
            pieces += order
            # all input triggers on the Sync queue: the ~0.65us per-trigger
            # issue cost paces the transfers so only a couple are in flight,
            # giving the first (critical) pieces near-exclusive bandwidth
            for dst, ext, lo, hi in pieces:
                nc.sync.dma_start(dst[:, lo:hi], ext.ap()[:, lo:hi])
            # dummy exps: the first pulls the ~2.7us ACT table load for Exp
            # off the critical path; the rest keep ScalarE busy so its clock
            # ramps up before the real exps start
            dume = small.tile([128, 512], F32)
            nc.scalar.activation(
                dume[:, 0:1], dume_in[:], mybir.ActivationFunctionType.Exp,
                bias=0.0, scale=1.0,
            )
            for _ in range(6):
                nc.scalar.activation(
                    dume[:], wsrc[:], mybir.ActivationFunctionType.Exp,
                    bias=0.0, scale=1.0,
                )

            def o_mms(k, fls, pT, j0=0, j1=None):
                """Flipped O-matmuls for tile k: P chunks stationary, Vaug
                (65 cols) moving. Chunk j accumulates into a [128, 65] psum
                region at a 128-col stride (4 per bank, none crossing a 2KB
                bank boundary); start=False always - the banks are claimed
                by per-pass dummy matmuls."""
                for j in range(j0, nblk if j1 is None else j1):
                    nc.tensor.matmul(
                        fls[j // 4][:, (j % 4) * 128 : (j % 4) * 128 + OW],
                        lhsT=pT[:, j * 128 : (j + 1) * 128],
                        rhs=va_s[:, VAW * k : VAW * k + OW],
                        start=False,
                        stop=(k == ntiles - 1),
                        skip_group_check=True,
                    )

            def epilogue(p, fls):
                """Pack the 8 [128, 65] chunk accumulators into two
                [128, 4*65] sbuf tiles and DMA straight out; copies split
                Scalar/Vector so both banks free up in parallel (the next
                pass's dummy claims wait on them), DMAs split across the
                two idle trigger queues."""
                off = p * PASSW
                for t in range(nblk // 4):
                    ot = fin.tile([128, 4 * OW], F32, name="otf", tag="otf")
                    src = fls[t][:, 0:512].rearrange(
                        "p (c x) -> p c x", c=4
                    )[:, :, 0:OW]
                    if t == 0:
                        nc.scalar.copy(ot[:], src)
                    else:
                        nc.vector.tensor_copy(ot[:], src)
                    dst = out_ext.ap()[
                        (off // 128) + t * 4 : (off // 128) + t * 4 + 4, :, :
                    ].rearrange("c p x -> p c x")
                    (nc.sync if t % 2 else nc.gpsimd).dma_start(
                        dst, ot[:].rearrange("p (c x) -> p c x", c=4)
                    )

            def claim(fls):
                """Claim the flip banks: zero all 512 cols, set all
                has_written bits (also the pass-boundary WAR point for the
                previous pass's epilogue copies)."""
                for t in range(nblk // 4):
                    nc.tensor.matmul(
                        fls[t][:],
                        lhsT=zt[:],
                        rhs=wsrc[:],
                        start=True,
                        stop=True,
                        skip_group_check=True,
                    )

            # One flat loop over all (pass, tile) pairs: the O-matmuls trail
            # the score matmuls by 2 tiles ACROSS pass boundaries, so the PE
            # never drains between passes. Lag 2 keeps the O-matmuls' exp
            # sems satisfied at dispatch (the PE's unsatisfied-dep wait queue
            # is only 4 deep; a blocked O-matmul would head-of-line stall
            # ready score matmuls behind it). A pass's epilogue + the next
            # pass's bank claims are emitted when its last tile's O-matmuls
            # drain from the lag queue - one tile before the next pass's
            # first O-matmuls need the banks.
            pend = []  # [(p, k, pT, fls)]
            fls = None
            for g in range(npass * ntiles):
                p, k = divmod(g, ntiles)
                off = p * PASSW
                if k == 0:
                    fls = [
                        psF.tile([128, 512], F32, name=f"fls{i}", tag=f"fls{i}")
                        for i in range(nblk // 4)
                    ]
                sps = psS.tile(
                    [128, PASSW], F32, name="sps", tag="sps",
                    padded_shape=[128, PASSW],
                )
                drain = pend[0] if len(pend) == 2 else None
                # interleave the drained tile's flip chunks AROUND the two
                # score matmuls: the PE has a single background weight
                # buffer, so each 512-col score stream can only hide ONE
                # pending flip LDWEIGHTS - splitting the chain 4+4 hides two
                for i in range(CPP):
                    nc.tensor.matmul(
                        sps[:, i * BCHUNK : (i + 1) * BCHUNK],
                        lhsT=kc_s[:, 128 * k : 128 * (k + 1)],
                        rhs=qc_s[:, off + i * BCHUNK : off + (i + 1) * BCHUNK],
                        start=True,
                        stop=True,
                    )
                    if drain is not None and i == 0:
                        o_mms(drain[1], drain[3], drain[2], 0, 4)
                pT = pTp.tile([128, PASSW], BF16, name="pT", tag="pT")
                # split exp: exact on ScalarE, bit-trick on VectorE
                nc.scalar.activation(
                    pT[:, 0:SCOL],
                    sps[:, 0:SCOL],
                    mybir.ActivationFunctionType.Exp,
                    bias=0.0,
                    scale=1.0,
                )
                nc.vector.tensor_scalar(
                    pT[:, SCOL:PASSW].bitcast(I16),
                    sps[:, SCOL:PASSW],
                    A_TRICK,
                    B_TRICK,
                    mybir.AluOpType.mult,
                    mybir.AluOpType.add,
                )
                if g == 0:
                    claim(fls)
                if drain is not None:
                    pp, kk, pTT, ffls = pend.pop(0)
                    o_mms(kk, ffls, pTT, 4, nblk)
                    if kk == ntiles - 1:
                        epilogue(pp, ffls)
                        claim(fls)
                pend.append((p, k, pT, fls))
            for pp, kk, pTT, ffls in pend:
                o_mms(kk, ffls, pTT)
                if kk == ntiles - 1:
                    epilogue(pp, ffls)

    _split_multi_waits(nc)
    return nc


_BUILD_CACHE: dict[int, object] = {}


def kernel(
    query,
    context,
    mem_keys,
    mem_values,
    mem_contexts,
    mem_timestamps,
    used_slots,
    _want_trace: bool = False,
):
    query = np.asarray(query, dtype=np.float32)
    context = np.asarray(context, dtype=np.float32)
    mem_keys = np.asarray(mem_keys, dtype=np.float32)
    mem_values = np.asarray(mem_values, dtype=np.float32)
    mem_contexts = np.asarray(mem_contexts, dtype=np.float32)
    mem_timestamps = np.asarray(mem_timestamps, dtype=np.float32)
    used_slots = np.asarray(used_slots).astype(bool)

    idx = np.flatnonzero(used_slots)
    count = idx.size
    if count == 0:
        # softmax over uniformly -1e9 scores is uniform over all M slots
        return np.broadcast_to(
            mem_values.mean(axis=0, dtype=np.float64).astype(np.float32), (B, D)
        ).copy()

    m_loc = max(128, int(math.ceil(count / (NCORES * 128))) * 128)
    m_tot = m_loc * NCORES
    ntiles = m_loc // 128

    # host-side layout prep: compact used slots, pad, shard, fuse operands.
    # kc row 96 carries the per-slot time-decay bias (pad rows: NEG_PAD);
    # qc row 96 is 1.0, so the score matmul emits fully-biased scores.
    kc = np.zeros((m_tot, KPAD), dtype=np.float32)
    kc[:count, :D] = mem_keys[idx]
    kc[:count, D:KDIM] = mem_contexts[idx]
    kc[:, KDIM] = NEG_PAD
    kc[:count, KDIM] = DECAY_COEF * np.exp(
        -TIME_WEIGHT * (CURRENT_TIME - mem_timestamps[idx])
    )
    va = np.zeros((m_tot, VAW), dtype=np.float32)
    va[:count, :D] = mem_values[idx]
    va[:, D] = 1.0

    qc = np.zeros((B, KPAD), dtype=np.float32)
    qc[:, :D] = query
    qc[:, D:KDIM] = 0.5 * context
    qc[:, KDIM] = 1.0
    qc_t = np.ascontiguousarray(qc.T).astype(ml_dtypes.bfloat16)

    in_maps = []
    for s in range(NCORES):
        lo, hi = s * m_loc, (s + 1) * m_loc
        va_tm = (
            va[lo:hi]
            .reshape(ntiles, 128, VAW)
            .transpose(1, 0, 2)
            .reshape(128, ntiles * VAW)
        )
        in_maps.append(
            {
                "qc_t": qc_t,
                "kc_t": np.ascontiguousarray(kc[lo:hi].T).astype(ml_dtypes.bfloat16),
                "vaug": np.ascontiguousarray(va_tm).astype(ml_dtypes.bfloat16),
            }
        )

    nc = _BUILD_CACHE.get(m_loc)
    if nc is None:
        nc = _build(m_loc)
        _BUILD_CACHE[m_loc] = nc

    res = bass_utils.run_bass_kernel_spmd(
        nc, in_maps, core_ids=list(range(NCORES)), trace=_want_trace
    )

    # host finale: sum the 8 cores' partial [B, 65] accumulators, divide by
    # the softmax denominator (column 64)
    acc = np.zeros((B, OW), dtype=np.float64)
    for s in range(NCORES):
        acc += res.results[s]["out"].reshape(B, OW)
    out = (acc[:, :D] / acc[:, D:]).astype(np.float32)
    if _want_trace:
        kernel.last_exec_time_ns = res.exec_time_ns
        kernel.last_results = res
    return out
